# revision 5
# baseline (speedup 1.0000x reference)
"""Transformer block (LN -> causal MHA -> residual -> LN -> GeLU FFN -> residual)
on 8 Trainium2 NeuronCores — zero-collective sequence-parallel version.

Sharding: DP=4 over batch x sequence-parallel 2 within each batch pair.
  Core c = (batch c//2, half s = c%2) owns 4 of the 8 128-token tiles:
    s=0 -> tiles {0,3,5,6}   (causal k-tile needs {1,4,6,7})
    s=1 -> tiles {1,2,4,7}   (causal k-tile needs {2,3,5,8})
  Both halves fit the static per-slot k-spans [2,4,6,8], so ONE SPMD program
  serves both; the causal masks and own-tile data arrive as per-core inputs.
  Each core duplicates LN1 + K/V projection for all 1024 tokens (cheaper
  than the 2x ReduceScatter of the Megatron split: +27us PE vs -82us
  collective), then computes Q/attention/proj/FFN for its own 512 tokens.
  No cross-core communication at all.

Precision:
  - Q/K/V projections and both FFN matmuls run in fp8 e4m3 with DoubleRow
    packing (2 contraction rows per partition -> 2x PE rate); weights are
    pre-scaled x256 host-side to clear the e4m3 subnormal region.  The K/Q
    descale rides the existing PSUM->SBUF bias copy (tensor_scalar mult+add);
    the V descale rides the softmax denominator (the ones column is 256);
    the FFN descales ride the gelu scale and the output copy.
  - Scores/AV/proj matmuls and the attention tensors stay bf16 (softmax
    precision); x is loaded bf16; scores accumulate in fp32 PSUM.
  - LayerNorm gains/biases are folded into the following weights host-side
    (W_eff = diag(g) W, b_eff = b + be W), so on-chip LN is just
    (x - mu) * rsqrt(var + eps); the V bias is applied post-softmax via
    bp_eff = bp + (bv + be1 Wv) @ Wp.
  - Residual path and LN stats stay fp32.

Causal softmax: S^T[k,q] per 128x128 tile (keys on partitions); exp on ACT
with the 1/8 scale folded in (|s/8| < ~3 so no max-subtraction); the last
two k-tiles of each slot get a multiplicative {0,1} bf16 mask; the
denominator rides the AV matmul as a 65th all-ones column of V.
"""

import sys

sys.path.insert(0, "/opt/trn_rl_repo")

import numpy as np
from contextlib import ExitStack

from concourse import bass, mybir, tile, bacc
from concourse.bass_utils import run_bass_kernel_spmd
from concourse.masks import make_identity

F32 = mybir.dt.float32
BF16 = mybir.dt.bfloat16
F8 = mybir.dt.float8e4

B, T, C = 4, 1024, 1024
H_ALL, D = 16, 64
FF = 4 * C
N_CORES = 8
NT = T // 128           # 8 token tiles
NTH = 4                 # own token tiles per core
CO = C // 128           # 8 feature chunks (contraction)
FCO = C // 128          # 8 qkv-feature chunks
HCO = FF // 128         # 32 hidden chunks
SLOTS = 4
KSPAN = [2, 4, 6, 8]    # static k-tile span per own-tile slot
OWN = [[0, 3, 5, 6], [1, 2, 4, 7]]  # own global tiles per half (slot order)
EPS = 1e-5
W8SCALE = 256.0         # host-side fp8 weight pre-scale

_PROG = None


def _build_program():
    nc = bacc.Bacc("TRN2", target_bir_lowering=False, debug=False)

    d_x = nc.dram_tensor("x", [128, NT, C], BF16, kind="ExternalInput").ap()
    d_xo = nc.dram_tensor("x_own", [128, NTH, C], BF16,
                          kind="ExternalInput").ap()
    d_xpb = nc.dram_tensor("x_pb", [128, NTH, C], F32,
                           kind="ExternalInput").ap()
    d_wq = nc.dram_tensor("wq", [128, CO, C], F8, kind="ExternalInput").ap()
    d_wk = nc.dram_tensor("wk", [128, CO, C], F8, kind="ExternalInput").ap()
    d_wv = nc.dram_tensor("wv", [128, CO, C], F8, kind="ExternalInput").ap()
    d_wp = nc.dram_tensor("wp", [128, CO, C], BF16, kind="ExternalInput").ap()
    d_w1 = nc.dram_tensor("w1", [128, CO, FF], F8, kind="ExternalInput").ap()
    d_w2 = nc.dram_tensor("w2", [128, HCO, C], F8, kind="ExternalInput").ap()
    d_bq = nc.dram_tensor("bq_pp", [128, FCO], F32, kind="ExternalInput").ap()
    d_bk = nc.dram_tensor("bk_pp", [128, FCO], F32, kind="ExternalInput").ap()
    d_b1 = nc.dram_tensor("b1_pp", [128, HCO], F32, kind="ExternalInput").ap()
    d_b2 = nc.dram_tensor("b2_row", [1, C], F32, kind="ExternalInput").ap()
    d_masks = nc.dram_tensor("masks", [128, SLOTS, 2, 128], BF16,
                             kind="ExternalInput").ap()
    d_out = nc.dram_tensor("out", [128, NTH, C], F32,
                           kind="ExternalOutput").ap()

    def bcast_row(dram_row, n, parts=128):
        return bass.AP(tensor=dram_row.tensor, offset=dram_row.offset,
                       ap=[[0, parts], [1, n]])

    with tile.TileContext(nc) as tc, ExitStack() as stack:
        con = stack.enter_context(tc.tile_pool(name="con", bufs=1))
        act = stack.enter_context(tc.tile_pool(name="act", bufs=1))
        p1 = stack.enter_context(tc.tile_pool(name="p1", bufs=2))

        # ---- first x tiles before the consts (gpsimd queue order) ----
        xts = []
        for t in range(2):
            x_t = p1.tile([128, C], BF16, tag="x_t", bufs=4, name=f"xt{t}")
            nc.gpsimd.dma_start(out=x_t[:], in_=d_x[:, t, :])
            xts.append(x_t)
        # ---- constants (gpsimd queue; weights go on sync/SP queue) ----
        ident = con.tile([128, 128], BF16)
        make_identity(nc, ident)
        epst = con.tile([128, 1], F32)
        nc.vector.memset(epst, EPS)
        b2r = con.tile([128, C], BF16)
        nc.gpsimd.dma_start(out=b2r[:], in_=bcast_row(d_b2, C))
        bq_pp = con.tile([128, FCO], F32)
        nc.gpsimd.dma_start(out=bq_pp[:], in_=d_bq[:])
        bk_pp = con.tile([128, FCO], F32)
        nc.gpsimd.dma_start(out=bk_pp[:], in_=d_bk[:])
        b1_pp = con.tile([128, HCO], F32)
        nc.gpsimd.dma_start(out=b1_pp[:], in_=d_b1[:])
        masks_sb = con.tile([128, SLOTS, 2, 128], BF16)
        nc.gpsimd.dma_start(out=masks_sb[:], in_=d_masks[:])
        ones_bf = con.tile([128, 64], BF16)
        nc.vector.memset(ones_bf, 1.0)

        def layernorm_tile(pool, src_ap, out_bf):
            """out_bf (bf16) = (src - mean) * rsqrt(var + eps)."""
            stats = pool.tile([128, 2, 6], F32, tag="ln_stats", bufs=3)
            nc.vector.bn_stats(out=stats[:, 0, :], in_=src_ap[:, 0:512])
            nc.vector.bn_stats(out=stats[:, 1, :], in_=src_ap[:, 512:1024])
            mv = pool.tile([128, 2], F32, tag="ln_mv", bufs=3)
            nc.vector.bn_aggr(out=mv[:], in_=stats[:])
            std = pool.tile([128, 1], F32, tag="ln_std", bufs=3)
            nc.scalar.activation(out=std[:], in_=mv[:, 1:2],
                                 func=mybir.ActivationFunctionType.Sqrt,
                                 bias=epst[:], scale=1.0)
            nc.vector.reciprocal(out=std[:], in_=std[:])
            with nc.allow_low_precision(reason="bf16 ln output"):
                nc.vector.tensor_scalar(out=out_bf, in0=src_ap,
                                        scalar1=mv[:, 0:1], scalar2=std[:],
                                        op0=mybir.AluOpType.subtract,
                                        op1=mybir.AluOpType.mult)

        # persistent activations (tagA is reused for gT in the FFN phase)
        ln1T = act.tile([128, CO, T], F8, name="ln1T")
        KT = act.tile([128, FCO, T], BF16, tag="tagA", name="KT")
        QT = act.tile([128, FCO, 512], BF16, name="QT")
        lnQ = act.tile([128, CO, 512], F8, name="lnQ")
        Vp = act.tile([128, NT, H_ALL, 65], BF16, name="Vp")
        attnT = act.tile([128, CO, 512], BF16, name="attnT")
        x1_sb = act.tile([128, NTH, C], F32, name="x1")
        ln2T = act.tile([128, CO, 512], F8, name="ln2T")
        # the V descale (weights are x256) rides the softmax denominator:
        # numerator picks up 256 from V, so the ones column is 256 too.
        nc.vector.memset(Vp[:, :, :, 64:65], W8SCALE)

        ph14 = stack.enter_context(ExitStack())
        with ExitStack() as ph1:
            wqkv = ph1.enter_context(tc.tile_pool(name="wqkv", bufs=1))
            wv_sb = wqkv.tile([128, CO, C], F8)
            nc.sync.dma_start(out=wv_sb[:], in_=d_wv[:])
            wk_sb = wqkv.tile([128, CO, C], F8)
            nc.sync.dma_start(out=wk_sb[:], in_=d_wk[:])
            wq_sb = wqkv.tile([128, CO, C], F8)
            nc.sync.dma_start(out=wq_sb[:], in_=d_wq[:])

            big = ph14.enter_context(
                tc.tile_pool(name="big", bufs=2, space="PSUM"))
            trp = ph14.enter_context(
                tc.tile_pool(name="trp", bufs=1, space="PSUM"))
            half = ph14.enter_context(
                tc.tile_pool(name="half", bufs=3, space="PSUM"))

            # ===== Phase 1: LN1 + transpose + V projection, per tile =====
            for t in range(NT):
                if t < 2:
                    x_t = xts[t]
                else:
                    x_t = p1.tile([128, C], BF16, tag="x_t", bufs=4)
                    nc.gpsimd.dma_start(out=x_t[:], in_=d_x[:, t, :])
                ln = p1.tile([128, C], BF16, tag="ln_out", bufs=3)
                layernorm_tile(p1, x_t[:], ln[:])
                ptr = trp.tile([128, 1024], BF16, tag="tr")
                for co in range(CO):
                    nc.tensor.transpose(ptr[:, co * 128:(co + 1) * 128],
                                        ln[:, co * 128:(co + 1) * 128],
                                        ident[:])
                with nc.allow_low_precision(reason="fp8 qkv input"):
                    nc.scalar.copy(
                        out=ln1T[:, :, t * 128:(t + 1) * 128],
                        in_=ptr[:].rearrange("p (a b) -> p a b", b=128))
                # V for this token tile: out [128 toks, 1024 vfeats] (x256)
                pv = big.tile([128, 1024], F32, tag="big")
                for fh in range(2):
                    for cp in range(4):
                        nc.tensor.matmul(
                            pv[:, fh * 512:(fh + 1) * 512],
                            ln1T[:, 2 * cp:2 * cp + 2, t * 128:(t + 1) * 128],
                            wv_sb[:, 2 * cp:2 * cp + 2,
                                  fh * 512:(fh + 1) * 512],
                            start=(cp == 0), stop=(cp == 3),
                            perf_mode=mybir.MatmulPerfMode.DoubleRow)
                nc.scalar.copy(
                    out=Vp[:, t, :, 0:64],
                    in_=pv[:].rearrange("p (h d) -> p h d", d=64))

            # own-tile LN1 again (per-core own positions) for the Q input
            for j in range(SLOTS):
                xo_t = p1.tile([128, C], BF16, tag="x_t", bufs=4)
                nc.gpsimd.dma_start(out=xo_t[:], in_=d_xo[:, j, :])
                lno = p1.tile([128, C], BF16, tag="ln_out", bufs=3)
                layernorm_tile(p1, xo_t[:], lno[:])
                ptr = trp.tile([128, 1024], BF16, tag="tr")
                for co in range(CO):
                    nc.tensor.transpose(ptr[:, co * 128:(co + 1) * 128],
                                        lno[:, co * 128:(co + 1) * 128],
                                        ident[:])
                with nc.allow_low_precision(reason="fp8 q input"):
                    nc.scalar.copy(
                        out=lnQ[:, :, j * 128:(j + 1) * 128],
                        in_=ptr[:].rearrange("p (a b) -> p a b", b=128))

            # ===== Phase 2: K and Q projections (all fco) =====
            dsc = 1.0 / W8SCALE
            for fco in range(FCO):
                pk = big.tile([128, 1024], F32, tag="big")
                for qb in range(2):
                    for cp in range(4):
                        nc.tensor.matmul(
                            pk[:, qb * 512:(qb + 1) * 512],
                            wk_sb[:, 2 * cp:2 * cp + 2,
                                  fco * 128:(fco + 1) * 128],
                            ln1T[:, 2 * cp:2 * cp + 2,
                                 qb * 512:(qb + 1) * 512],
                            start=(cp == 0), stop=(cp == 3),
                            perf_mode=mybir.MatmulPerfMode.DoubleRow)
                nc.vector.tensor_scalar(out=KT[:, fco, :], in0=pk[:],
                                        scalar1=dsc,
                                        scalar2=bk_pp[:, fco:fco + 1],
                                        op0=mybir.AluOpType.mult,
                                        op1=mybir.AluOpType.add)
                pq = half.tile([128, 512], F32, tag="half")
                for cp in range(4):
                    nc.tensor.matmul(
                        pq[:],
                        wq_sb[:, 2 * cp:2 * cp + 2,
                              fco * 128:(fco + 1) * 128],
                        lnQ[:, 2 * cp:2 * cp + 2, :],
                        start=(cp == 0), stop=(cp == 3),
                        perf_mode=mybir.MatmulPerfMode.DoubleRow)
                nc.vector.tensor_scalar(out=QT[:, fco, :], in0=pq[:],
                                        scalar1=dsc,
                                        scalar2=bq_pp[:, fco:fco + 1],
                                        op0=mybir.AluOpType.mult,
                                        op1=mybir.AluOpType.add)

        # weights for the later phases (SBUF freed by wqkv pool close)
        wff = stack.enter_context(tc.tile_pool(name="wff", bufs=1))
        wp_sb = wff.tile([128, CO, C], BF16)
        nc.sync.dma_start(out=wp_sb[:], in_=d_wp[:])
        w1_sb = wff.tile([128, CO, FF], F8)
        nc.sync.dma_start(out=w1_sb[:], in_=d_w1[:])
        w2_sb = wff.tile([128, HCO, C], F8)
        nc.sync.dma_start(out=w2_sb[:], in_=d_w2[:])

        # ===== Phase 3: attention, head by head =====
        for h in range(H_ALL):
            fco, hpo = h // 2, 64 * (h % 2)
            av = half.tile([128, 512], F32, tag="half")
            # slot groups: slots 0+1 share one PSUM tile / exp op (2+4
            # k-tiles = 768 cols); slots 2 and 3 are their own groups.
            for group in ([(0, 0), (1, 2)], [(2, 0)], [(3, 0)]):
                nplanes = sum(KSPAN[j] for j, _ in group)
                ps_s = big.tile([128, 1024], F32, tag="big")
                pt = p1.tile([128, NT, 128], BF16, tag="pt", bufs=3)
                for j, base in group:
                    for kc in range(KSPAN[j]):
                        nc.tensor.matmul(
                            ps_s[:, (base + kc) * 128:(base + kc + 1) * 128],
                            KT[hpo:hpo + 64, fco, kc * 128:(kc + 1) * 128],
                            QT[hpo:hpo + 64, fco, j * 128:(j + 1) * 128],
                            start=True, stop=True)
                with nc.allow_low_precision(reason="bf16 softmax"):
                    nc.scalar.activation(
                        out=pt[:, 0:nplanes, :],
                        in_=ps_s[:].rearrange("p (a b) -> p a b",
                                              b=128)[:, 0:nplanes, :],
                        func=mybir.ActivationFunctionType.Exp,
                        scale=0.125)
                    for j, base in group:
                        kj = KSPAN[j]
                        nc.vector.tensor_mul(
                            out=pt[:, base + kj - 2:base + kj, :],
                            in0=pt[:, base + kj - 2:base + kj, :],
                            in1=masks_sb[:, j, :, :])
                for j, base in group:
                    kj = KSPAN[j]
                    for kc in range(kj):
                        nc.tensor.matmul(
                            av[0:65, j * 128:(j + 1) * 128],
                            Vp[:, kc, h, :],
                            pt[:, base + kc, :],
                            start=(kc == 0), stop=(kc == kj - 1))
            rec = p1.tile([128, 512], BF16, tag="rec", bufs=3)
            with nc.allow_low_precision(reason="softmax denom"):
                nc.vector.reciprocal(out=rec[64:65, :], in_=av[64:65, :])
            pbc = half.tile([128, 512], F32, tag="half")
            nc.tensor.matmul(pbc[0:64, :], ones_bf[64:65, 0:64],
                             rec[64:65, :], start=True, stop=True)
            with nc.allow_low_precision(reason="bf16 attn"):
                nc.vector.tensor_copy(out=rec[0:64, :], in_=pbc[0:64, :])
                if hpo == 0:
                    nc.vector.tensor_mul(out=attnT[0:64, fco, :],
                                         in0=av[0:64, :], in1=rec[0:64, :])
                else:
                    atile = p1.tile([64, 512], BF16, tag="atile")
                    nc.vector.tensor_mul(out=atile[:],
                                         in0=av[0:64, :], in1=rec[0:64, :])
                    nc.gpsimd.dma_start(out=attnT[64:128, fco, :],
                                        in_=atile[:])

        # ===== Phase 4: attn projection + residual + LN2, per tile =====
        for j in range(SLOTS):
            xpb = p1.tile([128, C], F32, tag="xpb")
            nc.gpsimd.dma_start(out=xpb[:], in_=d_xpb[:, j, :])
            stats = p1.tile([128, 2, 6], F32, tag="ln_stats", bufs=3)
            for fh in range(2):
                pp = half.tile([128, 512], F32, tag="half")
                for co in range(CO):
                    nc.tensor.matmul(
                        pp[:],
                        attnT[:, co, j * 128:(j + 1) * 128],
                        wp_sb[:, co, fh * 512:(fh + 1) * 512],
                        start=(co == 0), stop=(co == CO - 1))
                nc.vector.tensor_add(
                    out=x1_sb[:, j, fh * 512:(fh + 1) * 512],
                    in0=pp[:], in1=xpb[:, fh * 512:(fh + 1) * 512])
                # LN2 stats for this half right away (shortens the chain)
                nc.vector.bn_stats(
                    out=stats[:, fh, :],
                    in_=x1_sb[:, j, fh * 512:(fh + 1) * 512])
            mv = p1.tile([128, 2], F32, tag="ln_mv", bufs=3)
            nc.vector.bn_aggr(out=mv[:], in_=stats[:])
            std = p1.tile([128, 1], F32, tag="ln_std", bufs=3)
            nc.scalar.activation(out=std[:], in_=mv[:, 1:2],
                                 func=mybir.ActivationFunctionType.Sqrt,
                                 bias=epst[:], scale=1.0)
            nc.vector.reciprocal(out=std[:], in_=std[:])
            ln2 = p1.tile([128, C], BF16, tag="ln_out", bufs=3)
            with nc.allow_low_precision(reason="bf16 ln output"):
                nc.vector.tensor_scalar(out=ln2[:], in0=x1_sb[:, j, :],
                                        scalar1=mv[:, 0:1], scalar2=std[:],
                                        op0=mybir.AluOpType.subtract,
                                        op1=mybir.AluOpType.mult)
            ptr = trp.tile([128, 1024], BF16, tag="tr")
            for co in range(CO):
                nc.tensor.transpose(ptr[:, co * 128:(co + 1) * 128],
                                    ln2[:, co * 128:(co + 1) * 128],
                                    ident[:])
            with nc.allow_low_precision(reason="fp8 ffn input"):
                nc.scalar.copy(
                    out=ln2T[:, :, j * 128:(j + 1) * 128],
                    in_=ptr[:].rearrange("p (a b) -> p a b", b=128))
            # x1 += b2 (after LN2 consumed x1); final residual uses this.
            # Runs on gpsimd: DVE is the busy engine in this phase.
            nc.gpsimd.tensor_add(out=x1_sb[:, j, :], in0=x1_sb[:, j, :],
                                 in1=b2r[:])

        # ===== Phase 5: FFN (fp8 DoubleRow), split by 256-token halves so
        # the first half starts right after LN2 of own tiles 0-1 and the
        # FFN2 of half A overlaps the FFN1 of half B =====
        ph14.close()
        gT = act.tile([128, HCO, 512], F8, tag="tagA", name="gT")
        with tc.tile_pool(name="ffp", bufs=1, space="PSUM") as ffp, \
             tc.tile_pool(name="p5", bufs=2) as p5:
            for hf in range(2):
                t0, t1 = hf * 256, (hf + 1) * 256
                accs = {}
                for jj in range(2):
                    for fh in range(2):
                        accs[(jj, fh)] = ffp.tile(
                            [128, 512], F32, tag=f"acc{jj}{fh}", bufs=1,
                            name=f"acc{hf}{jj}{fh}")
                for i in range(16):
                    for sub in range(2):
                        m = 2 * i + sub
                        ph = ffp.tile([128, 256], F32, tag="h1", bufs=4)
                        for jp in range(4):
                            nc.tensor.matmul(
                                ph[:],
                                w1_sb[:, 2 * jp:2 * jp + 2,
                                      m * 128:(m + 1) * 128],
                                ln2T[:, 2 * jp:2 * jp + 2, t0:t1],
                                start=(jp == 0), stop=(jp == 3),
                                perf_mode=mybir.MatmulPerfMode.DoubleRow)
                        with nc.allow_low_precision(reason="fp8 gelu"):
                            nc.scalar.activation(
                                out=gT[:, m, t0:t1], in_=ph[:],
                                func=mybir.ActivationFunctionType.Gelu,
                                bias=b1_pp[:, m:m + 1], scale=1.0 / W8SCALE)
                    for jj in range(2):
                        j = hf * 2 + jj
                        for fh in range(2):
                            nc.tensor.matmul(
                                accs[(jj, fh)][:],
                                gT[:, 2 * i:2 * i + 2,
                                   j * 128:(j + 1) * 128],
                                w2_sb[:, 2 * i:2 * i + 2,
                                      fh * 512:(fh + 1) * 512],
                                start=(i == 0), stop=(i == 15),
                                perf_mode=mybir.MatmulPerfMode.DoubleRow)
                for jj in range(2):
                    j = hf * 2 + jj
                    for fh in range(2):
                        tmp = p5.tile([128, 512], F32, tag="tmp")
                        if fh == 0:
                            nc.scalar.activation(
                                out=tmp[:], in_=accs[(jj, fh)][:],
                                func=mybir.ActivationFunctionType.Copy,
                                scale=1.0 / W8SCALE)
                        else:
                            nc.vector.tensor_scalar_mul(
                                out=tmp[:], in0=accs[(jj, fh)][:],
                                scalar1=1.0 / W8SCALE)
                        o0 = p5.tile([128, 512], F32, tag="out")
                        nc.vector.tensor_add(
                            out=o0[:], in0=tmp[:],
                            in1=x1_sb[:, j, fh * 512:(fh + 1) * 512])
                        nc.sync.dma_start(
                            out=d_out[:, j, fh * 512:(fh + 1) * 512],
                            in_=o0[:])

    nc.finalize()
    return nc


def get_program():
    global _PROG
    if _PROG is None:
        _PROG = _build_program()
    return _PROG


def _tile_tok(a):
    """[T, C] row-major -> [128, NT, C] token-tiled."""
    return np.ascontiguousarray(
        a.reshape(-1, 128, a.shape[-1]).transpose(1, 0, 2))


def _tile_w(w, n_co, dt):
    """[K, N] -> [128, n_co, N] with K = n_co*128 on (partition, co)."""
    return np.ascontiguousarray(
        w.reshape(n_co, 128, w.shape[-1]).transpose(1, 0, 2).astype(dt))


def make_in_maps(inputs):
    np_bf16 = mybir.dt.np(BF16)
    np_f8 = mybir.dt.np(F8)
    inp = {k: np.asarray(v, dtype=np.float32) for k, v in inputs.items()}

    g1 = inp["g1"][:, None]
    g2 = inp["g2"][:, None]
    wq_t = _tile_w(W8SCALE * (g1 * inp["Wq"]), CO, np_f8)
    wk_t = _tile_w(W8SCALE * (g1 * inp["Wk"]), CO, np_f8)
    wv_t = _tile_w(W8SCALE * (g1 * inp["Wv"]), CO, np_f8)
    wp_t = _tile_w(inp["Wp"], CO, np_bf16)
    w1_t = _tile_w(W8SCALE * (g2 * inp["W1"]), CO, np_f8)
    w2_t = _tile_w(W8SCALE * inp["W2"], HCO, np_f8)
    bq_eff = inp["bq"] + inp["be1"] @ inp["Wq"]
    bk_eff = inp["bk"] + inp["be1"] @ inp["Wk"]
    bv_eff = inp["bv"] + inp["be1"] @ inp["Wv"]
    b1_eff = inp["b1"] + inp["be2"] @ inp["W1"]
    # V bias is applied after softmax-normalization (sum p = 1), so it can
    # ride the projection: bp_eff = bp + bv_eff @ Wp.
    bp_eff = inp["bp"] + bv_eff @ inp["Wp"]
    bq_pp = np.ascontiguousarray(bq_eff.reshape(FCO, 128).T)
    bk_pp = np.ascontiguousarray(bk_eff.reshape(FCO, 128).T)
    b1_pp = np.ascontiguousarray(b1_eff.reshape(HCO, 128).T)

    in_maps = []
    for c in range(N_CORES):
        b, s = c // 2, c % 2
        own = OWN[s]
        xt = inp["x"][b].reshape(NT, 128, C)
        x_own = np.ascontiguousarray(xt[own].transpose(1, 0, 2))
        x_pb = np.ascontiguousarray(
            (xt[own] + bp_eff).transpose(1, 0, 2))
        masks = np.zeros((128, SLOTS, 2, 128), np.float32)
        for j in range(SLOTS):
            g = own[j]
            kj = KSPAN[j]
            for i in range(2):
                kc = kj - 2 + i
                k_idx = kc * 128 + np.arange(128)[:, None]
                q_idx = g * 128 + np.arange(128)[None, :]
                masks[:, j, i, :] = (k_idx <= q_idx).astype(np.float32)
        m = {
            "x": _tile_tok(inp["x"][b]).astype(np_bf16),
            "x_own": x_own.astype(np_bf16),
            "x_pb": x_pb,
            "wq": wq_t, "wk": wk_t, "wv": wv_t, "wp": wp_t,
            "w1": w1_t, "w2": w2_t,
            "bq_pp": bq_pp, "bk_pp": bk_pp, "b1_pp": b1_pp,
            "b2_row": inp["b2"].reshape(1, -1),
            "masks": masks.astype(np_bf16),
        }
        in_maps.append(m)
    return in_maps


def assemble_output(results):
    outs = []
    for b in range(B):
        full = np.empty((NT, 128, C), np.float32)
        for s in range(2):
            o = results[b * 2 + s]["out"]  # [128, NTH, C]
            full[OWN[s]] = o.transpose(1, 0, 2)
        outs.append(full.reshape(T, C))
    return np.stack(outs).astype(np.float32)


def kernel(**inputs):
    nc = get_program()
    in_maps = make_in_maps(inputs)
    res = run_bass_kernel_spmd(nc, in_maps, core_ids=list(range(N_CORES)))
    return assemble_output(res.results)
